# revision 59
# baseline (speedup 1.0000x reference)
"""Trainium2 Bass kernel for nn_CSLRTransformer (dense transformer, 8 cores).

Sharding: 4 batch elements x 2-way sequence split = 8 cores.
Core c handles batch b=c//2, half h=c%2 (tokens h*512..h*512+511).

K/V rank-order gather: each core computes K,V for its OWN 512 tokens only,
pair-AllGathers them (fp8), then loads BOTH slots back so k_t/v_ext hold
the full 1024-token K/V in global rank order on every core (attention is
key-permutation invariant). The residual stream x lives own-half only;
the conv head needs just a 12-token edge halo, gathered once after layer 7.

Numerics: the transformer body runs fp8e4m3 matmuls in DoubleRow perf mode
(2 stationary rows per PE cell, 0.5 cycles/moving-row = 4x bf16):
  - weight linears (QKV/out/FFN) pair adjacent 128-blocks of the
    contraction: lhsT [128,2,128], rhs [128,2,N].
  - scores contract dh=64 per head; the pair dim is (real block, zero
    block): q_t/k_t carry a 5th all-zero mo-block and the AP stride trick
    q_t[h*64:, mo:5:(4-mo), :] yields [64,2,N] pairs (real, zero).
  - context pairs adjacent 128-token blocks of the key axis.
Weight tensors are pre-scaled on the host into fp8's normal range and the
scales fold away for free: Q/K scales (64x each) fold into the softmax exp
scale; the V scale cancels in the softmax division (the denominator ones
columns in v_ext are memset to the same 64.0); FFN1's scale commutes with
ReLU (r1 is stored scaled, FFN2's evac descales); out/FFN2 descale via the
evac's scale operand. Residual stream and LN stats stay f32; matmuls
accumulate in f32 PSUM. The conv/fc head stays bf16 (cheap, and its error
does not average out).

Softmax: exp on ACT (the dominant ACT load); denominators ride as 64
replicated ones-columns on v_ext so the context matmul lands them on PSUM
partitions 64:127 -- reciprocal (DVE) + multiply (Pool) finish the
normalization without the old PE broadcast matmul. Linear evacuations are
spread across DVE/Pool via tensor_scalar (scale+bias in one op) to keep
ACT free for exp; LN keeps ln/exp (rstd = exp(-0.5*ln(var))) on ACT with
the stats transforms moved to DVE/Pool. The get_activation_tables patch
steers every used ACT function into the natural_log_exp_and_others set so
the whole transformer body runs on one function table.
"""
import numpy as np
import ml_dtypes

import concourse.bacc as bacc
import concourse.bass as bass
import concourse.mybir as mybir
import concourse.tile as tile
from concourse.bass_utils import run_bass_kernel_spmd

# Steer bass's greedy ACT-table assignment (first set containing the
# function) so ln/exp/identity all pick natural_log_exp_and_others: remove
# them from the sets that precede it in act_info.json order. Only the
# ASSIGNMENT view changes -- emitted act_func_set_ids still index the real
# act_info.json, and the chosen set genuinely contains these functions, so
# hardware behavior is correct; it just stops reloading tables every chain.
_real_get_act_tables = None


def _patched_get_act_tables(arch):
    tabs = dict(_real_get_act_tables(arch))
    target = "natural_log_exp_and_others"
    if target in tabs:
        steer = {a for a in tabs[target]
                 if a.name.lower() in ("ln", "exp", "identity", "square",
                                       "copy", "relu")}
        seen_target = False
        out = {}
        for name, s in tabs.items():
            if name == target:
                seen_target = True
            out[name] = s if seen_target else (s - steer)
        return out
    return tabs


def _install_act_table_patch():
    global _real_get_act_tables
    if _real_get_act_tables is None:
        _real_get_act_tables = bacc.get_activation_tables
        bacc.get_activation_tables = _patched_get_act_tables


_install_act_table_patch()

dt = mybir.dt
AF = mybir.ActivationFunctionType
ALU = mybir.AluOpType
DRow = mybir.MatmulPerfMode.DoubleRow

P = 128
B, T, IN_DIM, D, H, NCLS = 4, 1024, 231, 512, 8, 1296
NL, DFF, DH = 8, 2048, 64
TH = T // 2            # 512 own tokens
TP = TH // 2           # 256 own pooled positions
KIN = 256              # padded embed contraction (231 -> 256)
NCP = 1408             # padded classes (1296 -> 11*128)
EPS = 1e-5
F32 = dt.float32
F32R = dt.float32r
BF16 = dt.bfloat16
F8 = dt.float8e4

SW = 64.0              # scale for q/k/v/ff1 weights (activations stay scaled)
SB = 256.0             # scale for out/ff2 weights (descaled at evac)
EXP_SCALE = 0.125 / (SW * SW)
OUT_SCALE = 1.0 / SB
FF2_SCALE = 1.0 / (SW * SB)

_CACHE = {}


def _build(single_core=False):
    nc = bacc.Bacc("TRN2", target_bir_lowering=False, debug=False, num_devices=8)

    # ---- DRAM I/O ----
    poses_t = nc.dram_tensor("poses_t", [KIN, TH], F32R, kind="ExternalInput")
    pos_t = nc.dram_tensor("pos_t", [D, TH], F32, kind="ExternalInput")
    edges = nc.dram_tensor("edges", [P, 2], F32, kind="ExternalInput")
    emb_wt = nc.dram_tensor("emb_wt", [4, P, KIN], F32R, kind="ExternalInput")
    emb_b = nc.dram_tensor("emb_b", [D], F32, kind="ExternalInput")
    ln0_g = nc.dram_tensor("ln0_g", [D], F32, kind="ExternalInput")
    q_wt = nc.dram_tensor("q_wt", [NL, 4, P, D], F8, kind="ExternalInput")
    k_wt = nc.dram_tensor("k_wt", [NL, 4, P, D], F8, kind="ExternalInput")
    # double-quantized (hi, lo) fp8 weights: lo = fp8(w*scale - hi), both at
    # the same scale so they accumulate into one PSUM with one descale
    v_wt = nc.dram_tensor("v_wt", [NL, 2, 4, P, D], F8, kind="ExternalInput")
    out_wt = nc.dram_tensor("out_wt", [NL, 4, P, 2 * D], F8,
                            kind="ExternalInput")
    ff1_wt = nc.dram_tensor("ff1_wt", [NL, 16, P, 2 * D], F8,
                            kind="ExternalInput")
    ff2_wt = nc.dram_tensor("ff2_wt", [NL, 4, P, 2 * DFF], F8,
                            kind="ExternalInput")
    lbias = nc.dram_tensor("lbias", [NL, 32 * P], F32, kind="ExternalInput")
    c1_wt = nc.dram_tensor("c1_wt", [5, 4, P, D], BF16, kind="ExternalInput")
    bn1_s = nc.dram_tensor("bn1_s", [D], F32, kind="ExternalInput")
    bn1_t = nc.dram_tensor("bn1_t", [D], F32, kind="ExternalInput")
    c2_wt = nc.dram_tensor("c2_wt", [3, 4, P, D], BF16, kind="ExternalInput")
    bn2_s = nc.dram_tensor("bn2_s", [D], F32, kind="ExternalInput")
    bn2_t = nc.dram_tensor("bn2_t", [D], F32, kind="ExternalInput")
    fc1_wt = nc.dram_tensor("fc1_wt", [2, P, D], BF16, kind="ExternalInput")
    fc1_b = nc.dram_tensor("fc1_b", [D // 2], F32, kind="ExternalInput")
    fc2_wt = nc.dram_tensor("fc2_wt", [11, P, D // 2], BF16, kind="ExternalInput")
    fc2_b = nc.dram_tensor("fc2_b", [NCP], F32, kind="ExternalInput")
    out_d = nc.dram_tensor("out", [NCP, TP], F32, kind="ExternalOutput")

    with tile.TileContext(nc) as tc:
        with (
            tc.tile_pool(name="state", bufs=1) as state,
            tc.tile_pool(name="act1", bufs=1) as act1,
            tc.tile_pool(name="act2", bufs=3) as act2,
            tc.tile_pool(name="wts", bufs=3) as wts,
            tc.tile_pool(name="wvp", bufs=2) as wvp,
            tc.tile_pool(name="kqp", bufs=2) as kqp,
            tc.tile_pool(name="ff1p", bufs=1) as ff1p,
            tc.tile_pool(name="ff2p", bufs=1) as ff2p,
            tc.tile_pool(name="biasp", bufs=2) as biasp,
            tc.tile_pool(name="headp", bufs=1) as headp,
            tc.tile_pool(name="ps_mm", bufs=2, space="PSUM") as ps_mm,
            tc.tile_pool(name="ps_ctx", bufs=2, space="PSUM") as ps_ctx,
            tc.tile_pool(name="ps_sc", bufs=2, space="PSUM") as ps_sc,
            tc.tile_pool(name="dram", bufs=2, space="DRAM") as dram,
        ):
            # ---------- constants / persistent ----------
            ones_f32 = state.tile([P, P], F32)
            nc.vector.memset(ones_f32[:], 1.0)
            ones_sq = state.tile([P, P], F32R)
            nc.vector.tensor_copy(ones_sq[:], ones_f32[:])
            ones_bf = state.tile([P, P], BF16)
            nc.vector.tensor_copy(ones_bf[:], ones_f32[:])
            x_sb = state.tile([P, 4, TH], F32R)      # residual stream (own)
            xblk = state.tile([P, 4, TH], F32R)      # block-residual save
            # [ktok, kt, head, dh | 64 ones(=SW) denominator columns]: the
            # ones value matches the V weight scale so the softmax division
            # cancels it exactly
            v_ext = state.tile([P, 8, H, 128], F8)
            nc.vector.memset(v_ext[:, :, :, 64:128], SW)
            # q_t/k_t mo-blocks 0-3 hold data; block 4 is the all-zero
            # DoubleRow pair for the 64-deep score contraction
            q_t = state.tile([P, 5, TH], F8)
            nc.vector.memset(q_t[:, 4, :], 0.0)
            k_t = state.tile([P, 5, T], F8)
            nc.vector.memset(k_t[:, 4, :], 0.0)
            epsb = state.tile([P, 1], F32)
            nc.vector.memset(epsb[:], EPS)

            def load_pcol(dr, n, eng=None):
                # [n*128] dram vector -> [128, n] sbuf (d on partitions)
                t_ = state.tile([P, n], F32, tag=f"b{n}_{dr.tensor.name}")
                (eng or nc.sync).dma_start(t_[:],
                                           dr.rearrange("(o p) -> p o", p=P))
                return t_

            emb_b_sb = load_pcol(emb_b.ap(), 4)
            ln0g_sb = load_pcol(ln0_g.ap(), 4)
            bn1s_sb = load_pcol(bn1_s.ap(), 4)
            bn1t_sb = load_pcol(bn1_t.ap(), 4)
            bn2s_sb = load_pcol(bn2_s.ap(), 4)
            bn2t_sb = load_pcol(bn2_t.ap(), 4)
            fc1b_sb = load_pcol(fc1_b.ap(), 2)
            fc2b_sb = load_pcol(fc2_b.ap(), 11)
            edges_sb = state.tile([P, 2], F32)
            nc.sync.dma_start(edges_sb[:], edges[:])

            # ---------- helpers ----------
            def linear(x, w_dram, nk, nm, ncols, evac, out, wdt=F8,
                       wtiles=None, dr=True, dbl=False):
                """out[:, mo, :ncols] = w.T @ x (DoubleRow fp8 by default).
                w_dram: mo-blocked AP [nm, 128, nk*128] (or [..., 2*nk*128]
                hi/lo when dbl); wtiles: preloaded 3D weight tiles."""
                pp = [None, 0]
                nrep = 2 if dbl else 1
                for mo in range(nm):
                    if wtiles is not None:
                        wt = wtiles[:, mo]
                    else:
                        shape = [P, 2, nk, P] if dbl else [P, nk, P]
                        wt = wts.tile(shape, wdt, tag="wmo")
                        eng = nc.gpsimd if mo % 2 == 0 else nc.sync
                        eng.dma_start(wt[:], w_dram[mo])
                    for nti in range((ncols + 511) // 512):
                        cs = min(512, ncols - nti * 512)
                        if pp[0] is None or pp[1] == 1:
                            ppt = ps_sc.tile([P, 2, 512], F32, tag="psp")
                            pp[0] = ppt
                            pp[1] = 0
                        ps = pp[0][:, pp[1], :]
                        pp[1] += 1
                        if dr:
                            for rep in range(nrep):
                                wr = wt[:, rep] if dbl else wt
                                for kp in range(nk // 2):
                                    nc.tensor.matmul(
                                        ps[:, :cs],
                                        wr[:, 2 * kp:2 * kp + 2, :],
                                        x[:, 2 * kp:2 * kp + 2,
                                          nti * 512:nti * 512 + cs],
                                        start=(rep == 0 and kp == 0),
                                        stop=(rep == nrep - 1
                                              and kp == nk // 2 - 1),
                                        perf_mode=DRow)
                        else:
                            for ko in range(nk):
                                nc.tensor.matmul(
                                    ps[:, :cs],
                                    wt[:, ko, :],
                                    x[:, ko, nti * 512:nti * 512 + cs],
                                    start=(ko == 0), stop=(ko == nk - 1))
                        evac(ps[:, :cs], mo, nti * 512, out)
                return out

            def warm_pre(n):
                # dependency-free matmuls that keep the PE p-state hot while
                # the preceding elementwise chain (residual adds etc.) runs
                wps = ps_mm.tile([P, 512], F32, tag="ps")
                for _ in range(n):
                    nc.tensor.matmul(wps[:, 0:P], ones_bf[:], ones_bf[:],
                                     start=True, stop=True)

            def preload(eng, w_dram, nm, nk, pool, tag, wdt=F8, dbl=False):
                """Whole weight matrix in ONE merged DMA: tile
                [P, nm, (2,) nk, P] <- dram [nm, P, (2*)nk*P]. Double-
                buffered pools make the DMA wait-free, so it never clogs
                its HWDGE ring."""
                shape = ([P, nm, 2, nk, P] if dbl else [P, nm, nk, P])
                wt = pool.tile(shape, wdt, tag=tag)
                eng.dma_start(wt[:], w_dram.rearrange("m p z -> p m z"))
                return wt

            def evac_bias(bias_sb, func=AF.Identity, scale=1.0):
                def _e(ps, mo, c0, out):
                    nc.scalar.activation(
                        out[:, mo, c0:c0 + ps.shape[-1]], ps,
                        func, bias=bias_sb[:, mo:mo + 1], scale=scale)
                return _e

            def evac_bias_ts(eng, bias_sb, scale=None):
                # (ps * scale + bias) on DVE/Pool in one tensor_scalar
                def _e(ps, mo, c0, out):
                    o = out[:, mo, c0:c0 + ps.shape[-1]]
                    if scale is None:
                        eng.tensor_scalar(o, ps, bias_sb[:, mo:mo + 1],
                                          None, ALU.add)
                    else:
                        eng.tensor_scalar(o, ps, scale,
                                          bias_sb[:, mo:mo + 1],
                                          ALU.mult, ALU.add)
                return _e

            def evac_relu_ts(eng, bias_sb):
                def _e(ps, mo, c0, out):
                    eng.tensor_scalar(
                        out[:, mo, c0:c0 + ps.shape[-1]], ps,
                        bias_sb[:, mo:mo + 1], 0.0, ALU.add, ALU.max)
                return _e

            def evac_mix(*fns):
                def _e(ps, mo, c0, out):
                    fns[mo % len(fns)](ps, mo, c0, out)
                return _e

            def ln_part(x, out, c0, cs, gamma=None, warm=0):
                """LayerNorm over columns [c0, c0+cs): stats via ones-matmul,
                transforms spread over DVE/Pool, rstd = exp(-0.5*ln(var)) on
                ACT (shared softmax table). `warm` extra matmuls re-fill ps1
                after its readers finish, keeping the PE p-state hot through
                the elementwise chain."""
                sl = slice(c0, c0 + cs)
                sq = act1.tile([P, 4, 512], F32R, tag="sq")
                ps1 = ps_mm.tile([P, 512], F32, tag="ps")
                for ko in range(4):
                    nc.tensor.matmul(ps1[:, :cs], ones_sq[:], x[:, ko, sl],
                                     start=(ko == 0), stop=(ko == 3))
                for ko in range(4):
                    nc.vector.tensor_tensor(sq[:, ko, sl], x[:, ko, sl],
                                            x[:, ko, sl], ALU.mult)
                ps2 = ps_mm.tile([P, 512], F32, tag="ps")
                for ko in range(4):
                    nc.tensor.matmul(ps2[:, :cs], ones_sq[:], sq[:, ko, sl],
                                     start=(ko == 0), stop=(ko == 3))
                m2 = act1.tile([P, 512], F32, tag="m2")
                va = act1.tile([P, 512], F32, tag="va")
                r = act1.tile([P, 512], F32, tag="r")
                m = act1.tile([P, 512], F32, tag="m")
                # m2 = (S1/D)^2 on ACT in parallel with m = S1/D on DVE;
                # then ONE DVE stt: va = S2/D - m2 (EPS folds into Ln bias)
                nc.scalar.activation(m2[:, :cs], ps1[:, :cs], AF.Square,
                                     scale=1.0 / D)
                nc.vector.tensor_scalar(m[:, :cs], ps1[:, :cs], 1.0 / D,
                                        None, ALU.mult)
                nc.vector.scalar_tensor_tensor(
                    va[:, :cs], ps2[:, :cs], 1.0 / D, m2[:, :cs],
                    ALU.mult, ALU.subtract)
                # rstd = exp(-0.5*ln(var + EPS))
                nc.scalar.activation(va[:, :cs], va[:, :cs], AF.Ln,
                                     bias=epsb[:, 0:1])
                nc.scalar.activation(r[:, :cs], va[:, :cs], AF.Exp,
                                     scale=-0.5)
                for i in range(warm):
                    nc.tensor.matmul(ps1[:, :cs], ones_sq[:],
                                     x[:, i % 4, sl], start=True, stop=True)
                for ko in range(4):
                    eng = nc.vector if ko % 2 == 0 else nc.gpsimd
                    eng.tensor_tensor(sq[:, ko, sl], x[:, ko, sl],
                                      m[:, :cs], ALU.subtract)
                    eng.tensor_tensor(out[:, ko, sl], sq[:, ko, sl],
                                      r[:, :cs], ALU.mult)
                    if gamma is not None:
                        eng.tensor_scalar(
                            out[:, ko, sl], out[:, ko, sl],
                            gamma[:, ko:ko + 1], None, ALU.mult)

            def ln_chain(x, out, gamma=None):
                ln_part(x, out, 0, 512, gamma=gamma)

            def linear_chunk(x, wtiles, nk, nm, c0, cs, evac, out, dbl=False,
                             mos=None):
                """Column-range piece of a linear from preloaded weights."""
                pp = [None, 0]
                nrep = 2 if dbl else 1
                for mo in (mos if mos is not None else range(nm)):
                    if pp[0] is None or pp[1] == 1:
                        ppt = ps_sc.tile([P, 2, 512], F32, tag="psp")
                        pp[0] = ppt
                        pp[1] = 0
                    ps = pp[0][:, pp[1], :]
                    pp[1] += 1
                    for rep in range(nrep):
                        wr = wtiles[:, mo, rep] if dbl else wtiles[:, mo]
                        for kp in range(nk // 2):
                            nc.tensor.matmul(
                                ps[:, :cs], wr[:, 2 * kp:2 * kp + 2, :],
                                x[:, 2 * kp:2 * kp + 2, c0:c0 + cs],
                                start=(rep == 0 and kp == 0),
                                stop=(rep == nrep - 1 and kp == nk // 2 - 1),
                                perf_mode=DRow)
                    evac(ps[:, :cs], mo, c0, out)

            # ---------- embed + LN0 + pos ----------
            poses_sb = act1.tile([P, 2, TH], F32R, tag="qt")
            for ko in range(2):
                nc.gpsimd.dma_start(poses_sb[:, ko, :],
                                    poses_t[ko * P:(ko + 1) * P, :])
            xe = act1.tile([P, 4, TH], F32R, tag="r1")
            linear(poses_sb, emb_wt.ap(), 2, 4, TH, evac_bias(emb_b_sb), xe,
                   wdt=F32R, dr=False)
            ln_chain(xe, x_sb, gamma=ln0g_sb)
            for ko in range(4):
                nc.gpsimd.dma_start(xblk[:, ko, :],
                                    pos_t[ko * P:(ko + 1) * P, :])
            for ko in range(4):
                nc.vector.tensor_tensor(x_sb[:, ko, :], x_sb[:, ko, :],
                                        xblk[:, ko, :], ALU.add)

            # ---------- transformer layers ----------
            def prefetch_qkv(li):
                # merged single-DMA weight prefetch on the scalar/vector
                # rings; the sync ring is reserved for the gather chain
                kw = preload(nc.scalar, k_wt[li], 4, 4, kqp, "kw")
                qw = preload(nc.scalar, q_wt[li], 4, 4, kqp, "qw")
                wv = wvp.tile([P, 2, 4, 512], F8, tag="wv")
                nc.scalar.dma_start(
                    wv[:], v_wt[li].rearrange("r k p d -> p r k d"))
                return kw, qw, wv

            nxt = prefetch_qkv(0)
            for li in range(NL):
                kw_t, qw_t, wv = nxt
                # per-layer biases: ONE merged DMA -> [128, 32] column tile
                lb = biasp.tile([P, 32], F32, tag="lb")
                nc.scalar.dma_start(lb[:],
                                    lbias[li].rearrange("(o p) -> p o", p=P))
                qkb_sb = lb[:, 0:8]
                outb_sb = lb[:, 8:12]
                ff1b_sb = lb[:, 12:28]
                ff2b_sb = lb[:, 28:32]
                # FFN weights for this layer (first use ~15us in)
                ff1w_t = preload(nc.scalar, ff1_wt[li], 16, 4, ff1p, "ff1w",
                                 dbl=True)
                ff2w_t = preload(nc.scalar, ff2_wt[li], 4, 16, ff2p, "ff2w",
                                 dbl=True)
                outw_t = preload(nc.scalar, out_wt[li], 4, 4, ff2p, "outw",
                                 dbl=True)

                # LN1 -> K (K matmuls chase the LN chain); K is emitted
                # mo-group-first (mo 0/1 over both column chunks, then 2/3)
                # so the first gather launches as early as possible
                warm_pre(6)
                h1 = act1.tile([P, 4, TH], F8, tag="h1")
                b_ik1 = dram.tile([2 * P, TH], F8, tag="kin1")
                b_ok1 = dram.tile([2, 2 * P, TH], F8, tag="kout1")
                b_ik2 = dram.tile([2 * P, TH], F8, tag="kin2")
                b_ok2 = dram.tile([2, 2 * P, TH], F8, tag="kout2")
                evk = evac_mix(evac_bias_ts(nc.vector, qkb_sb[:, 4:]),
                               evac_bias(qkb_sb[:, 4:]))

                def k_gather(b_ik_, b_ok_, koff):
                    # entire chain on the sync HWDGE ring, in order
                    bik_ = b_ik_.rearrange("(ko p) t -> p ko t", p=P)
                    nc.sync.dma_start(bik_[:, 0:2, :],
                                      k_t[:, koff:koff + 2, 0:TH])
                    if single_core:
                        nc.sync.dma_start(b_ok_[0], b_ik_[:])
                        nc.sync.dma_start(b_ok_[1], b_ik_[:])
                    else:
                        nc.gpsimd.collective_compute(
                            "AllGather", ALU.bypass,
                            ins=[b_ik_.opt()], outs=[b_ok_.opt()],
                            replica_groups=[[0, 1], [2, 3], [4, 5], [6, 7]])
                    for slot in range(2):
                        nc.sync.dma_start(
                            k_t[:, koff:koff + 2, slot * TH:(slot + 1) * TH],
                            b_ok_[slot].rearrange("(ko p) t -> p ko t", p=P))

                ln_part(x_sb, h1, 0, 256, warm=6)
                linear_chunk(h1, kw_t, 4, 4, 0, 256, evk, k_t, mos=(0, 1))
                ln_part(x_sb, h1, 256, 256)
                linear_chunk(h1, kw_t, 4, 4, 256, 256, evk, k_t, mos=(0, 1))
                k_gather(b_ik1, b_ok1, 0)
                linear_chunk(h1, kw_t, 4, 4, 0, 256, evk, k_t, mos=(2, 3))
                linear_chunk(h1, kw_t, 4, 4, 256, 256, evk, k_t, mos=(2, 3))
                k_gather(b_ik2, b_ok2, 2)

                # Q own (feeds scores as soon as the gather lands)
                evq = evac_mix(evac_bias_ts(nc.vector, qkb_sb[:, 0:]),
                               evac_bias(qkb_sb[:, 0:]))
                linear(h1, None, 4, 4, TH, evq, q_t, wtiles=qw_t)

                # V own (token-major via stationary-activation trick)
                v_stg = act1.tile([P, 4, H, DH], F8, tag="vstg")
                for tt in range(4):
                    ps = ps_mm.tile([P, 512], F32, tag="ps")
                    for rep in range(2):
                        for kp in range(2):
                            nc.tensor.matmul(
                                ps[:],
                                h1[:, 2 * kp:2 * kp + 2, tt * P:(tt + 1) * P],
                                wv[:, rep, 2 * kp:2 * kp + 2, :],
                                start=(rep == 0 and kp == 0),
                                stop=(rep == 1 and kp == 1),
                                perf_mode=DRow)
                    nc.vector.tensor_copy(
                        v_stg[:, tt],
                        ps[:].rearrange("p (h d) -> p h d", d=DH))
                b_iv = dram.tile([TH, D], F8, tag="vin")
                b_ov = dram.tile([2, TH, D], F8, tag="vout")
                nc.sync.dma_start(
                    b_iv.rearrange("(tt p) (h d) -> p tt h d", p=P, d=DH),
                    v_stg[:])
                if single_core:
                    nc.sync.dma_start(b_ov[0], b_iv[:])
                    nc.sync.dma_start(b_ov[1], b_iv[:])
                else:
                    nc.gpsimd.collective_compute(
                        "AllGather", ALU.bypass,
                        ins=[b_iv.opt()], outs=[b_ov.opt()],
                        replica_groups=[[0, 1], [2, 3], [4, 5], [6, 7]])
                for slot in range(2):
                    bovr = b_ov[slot].rearrange("(tt p) (h d) -> p tt h d",
                                                p=P, d=DH)
                    for tt in range(4):
                        # split across both HWDGE rings (scalar ring is idle
                        # during attention)
                        eng = nc.sync if tt % 2 == 0 else nc.scalar
                        eng.dma_start(
                            v_ext[:, slot * 4 + tt, :, 0:DH], bovr[:, tt])

                # pre-add the out-proj bias to the residual stream now (x is
                # idle through attention); the out evac then fuses the
                # residual add via scalar_tensor_tensor
                for ko in range(4):
                    nc.gpsimd.tensor_scalar(x_sb[:, ko, :], x_sb[:, ko, :],
                                            outb_sb[:, ko:ko + 1], None,
                                            ALU.add)

                if li == NL - 1:
                    c1_tiles = [preload(nc.scalar, c1_wt[k], 4, 4, headp,
                                        f"c1_{k}", wdt=BF16)
                                for k in range(5)]
                    c2_tiles = [preload(nc.scalar, c2_wt[k], 4, 4, headp,
                                        f"c2_{k}", wdt=BF16)
                                for k in range(3)]

                # attention, kt-phased: slot-0 scores for every head pair
                # first, so slot-1's gather latency hides behind them.
                # scores: DoubleRow with the (real, zero-block) pair trick;
                # context: DoubleRow over adjacent key-token blocks, with the
                # denominator riding on psum partitions 64:127 via the 64
                # ones-columns of v_ext.
                ctx = act1.tile([P, 4, TH], F8, tag="ctx")
                for mo in range(4):
                    p_t = act2.tile([P, 8, 2, TH], F8, tag="pt")
                    st = 4 - mo
                    for kt in range(8):
                        pp = ps_sc.tile([P, 2, 512], F32, tag="psp")
                        for hh in range(2):
                            nc.tensor.matmul(
                                pp[:, hh, :],
                                k_t[hh * DH:(hh + 1) * DH, mo:5:st,
                                    kt * P:(kt + 1) * P],
                                q_t[hh * DH:(hh + 1) * DH, mo:5:st, :],
                                start=True, stop=True, perf_mode=DRow)
                        nc.scalar.activation(p_t[:, kt], pp[:], AF.Exp,
                                             scale=EXP_SCALE)
                    for hh in range(2):
                        h = 2 * mo + hh
                        bp = hh * 64
                        psc = ps_ctx.tile([P, 512], F32, tag="ps_ctx")
                        for tp in range(4):
                            nc.tensor.matmul(
                                psc[:],
                                v_ext[:, 2 * tp:2 * tp + 2, h, :],
                                p_t[:, 2 * tp:2 * tp + 2, hh, :],
                                start=(tp == 0), stop=(tp == 3),
                                perf_mode=DRow)
                        rcp = act1.tile([64, TH], F32R, tag="rcp")
                        with nc.allow_low_precision(reason="softmax denom"):
                            nc.vector.reciprocal(rcp[:], psc[64:128, :])
                        nc.vector.tensor_tensor(
                            ctx[bp:bp + 64, mo, :], psc[0:64, :],
                            rcp[:], ALU.mult)

                # prefetch next layer's K/Q/V weights while attention runs
                if li + 1 < NL:
                    nxt = prefetch_qkv(li + 1)

                # out-proj: evac fuses the residual add (bias was pre-added
                # to x during attention): x += psum * OUT_SCALE, one DVE op
                def evac_resid(scale):
                    def _e(ps, mo, c0, out):
                        nc.vector.scalar_tensor_tensor(
                            out[:, mo, c0:c0 + ps.shape[-1]], ps, scale,
                            out[:, mo, c0:c0 + ps.shape[-1]],
                            ALU.mult, ALU.add)
                    return _e

                linear(ctx, None, 4, 4, TH, evac_resid(OUT_SCALE), x_sb,
                       wtiles=outw_t, dbl=True)

                # FFN; LN2 -> FFN1 in 256-col chunks; ff2 bias pre-added to
                # x during the ff1 phase, ff2 evac fuses the residual add
                h2 = act1.tile([P, 4, TH], F8, tag="h1")
                r1 = act1.tile([P, 16, TH], F8, tag="r1")
                warm_pre(6)
                evf = evac_mix(evac_relu_ts(nc.vector, ff1b_sb),
                               evac_bias(ff1b_sb, func=AF.Relu))
                for c0 in (0, 256):
                    ln_part(x_sb, h2, c0, 256, warm=(6 if c0 == 0 else 0))
                    linear_chunk(h2, ff1w_t, 4, 16, c0, 256, evf, r1,
                                 dbl=True)
                for ko in range(4):
                    nc.gpsimd.tensor_scalar(x_sb[:, ko, :], x_sb[:, ko, :],
                                            ff2b_sb[:, ko:ko + 1], None,
                                            ALU.add)
                linear(r1, None, 16, 4, TH, evac_resid(FF2_SCALE), x_sb,
                       wtiles=ff2w_t, dbl=True)

                # block residual: y = block(y) + y at layers 3, 5, 7
                if li in (3, 5, 7):
                    for ko in range(4):
                        nc.vector.tensor_tensor(x_sb[:, ko, :],
                                                x_sb[:, ko, :],
                                                xblk[:, ko, :], ALU.add)
                if li in (1, 3, 5):
                    for ko in range(4):
                        nc.gpsimd.tensor_copy(xblk[:, ko, :], x_sb[:, ko, :])

            # ---------- conv-edge halo gather (12 raw tokens) ----------
            b_ie = dram.tile([D, 12], F32R, tag="egin")
            b_oe = dram.tile([2, D, 12], F32R, tag="egout")
            bie = b_ie.rearrange("(ko p) t -> p ko t", p=P)
            nc.sync.dma_start(bie[:, :, 0:6], x_sb[:, :, 0:6])
            nc.sync.dma_start(bie[:, :, 6:12], x_sb[:, :, 506:512])
            if single_core:
                nc.gpsimd.dma_start(b_oe[0], b_ie[:])
                nc.sync.dma_start(b_oe[1], b_ie[:])
            else:
                nc.gpsimd.collective_compute(
                    "AllGather", ALU.bypass,
                    ins=[b_ie.opt()], outs=[b_oe.opt()],
                    replica_groups=[[0, 1], [2, 3], [4, 5], [6, 7]])
            # pool the interior while the edge gather is in flight
            xpe = act1.tile([P, 4, 262], BF16, tag="xpe")
            nc.vector.tensor_tensor(xpe[:, :, 3:259], x_sb[:, :, 0:TH:2],
                                    x_sb[:, :, 1:TH:2], ALU.add)
            s0 = act1.tile([P, 4, 12], F32R, tag="s0")
            s1 = act1.tile([P, 4, 12], F32R, tag="s1")
            nc.sync.dma_start(s0[:], b_oe[0].rearrange("(ko p) t -> p ko t", p=P))
            nc.sync.dma_start(s1[:], b_oe[1].rearrange("(ko p) t -> p ko t", p=P))
            pe12 = act1.tile([P, 4, 12], F32R, tag="pe12")
            nc.vector.tensor_tensor(pe12[:], s0[:], s1[:], ALU.add)
            nc.vector.tensor_tensor(pe12[:, :, 0:6], pe12[:, :, 0:6],
                                    x_sb[:, :, 0:6], ALU.subtract)
            nc.vector.tensor_tensor(pe12[:, :, 6:12], pe12[:, :, 6:12],
                                    x_sb[:, :, 506:512], ALU.subtract)
            ph = act1.tile([P, 4, 6], F32, tag="ph")
            nc.vector.tensor_tensor(ph[:], pe12[:, :, 0:12:2],
                                    pe12[:, :, 1:12:2], ALU.add)

            # ---------- head: pool -> conv1 -> conv2 -> fc1 -> fc2 ----------
            # (avg-pool(2) interior already computed above; the 0.5 factor is
            # folded into conv1 weights)
            # halo: ph[3:6]=peer high edge (left halo), ph[0:3]=peer low (right)
            nc.vector.tensor_scalar(xpe[:, :, 0:3], ph[:, :, 3:6],
                                    edges_sb[:, 0:1], None, ALU.mult)
            nc.vector.tensor_scalar(xpe[:, :, 259:262], ph[:, :, 0:3],
                                    edges_sb[:, 1:2], None, ALU.mult)

            def conv_block(src, ntaps, wtiles, ncols, bn_s, bn_t, out):
                # wtiles: per-tap merged [128, 4(mo), 4(ko), 128]
                for mo in range(4):
                    ps = ps_mm.tile([P, 512], F32, tag="ps")
                    for k in range(ntaps):
                        for ko in range(4):
                            nc.tensor.matmul(
                                ps[:, 0:ncols], wtiles[k][:, mo, ko, :],
                                src[:, ko, k:k + ncols],
                                start=(k == 0 and ko == 0),
                                stop=(k == ntaps - 1 and ko == 3))
                    nc.scalar.activation(out[:, mo, :], ps[:, 0:ncols],
                                         AF.Gelu, bias=bn_t[:, mo:mo + 1],
                                         scale=bn_s[:, mo:mo + 1])

            y1e = act1.tile([P, 4, 258], BF16, tag="ysb")
            conv_block(xpe, 5, c1_tiles, 258, bn1s_sb, bn1t_sb, y1e)
            # conv2 zero-pads at the GLOBAL sequence edges: kill the computed
            # y1 halo column on the outer side of each boundary core
            nc.vector.tensor_scalar(y1e[:, :, 0:1], y1e[:, :, 0:1],
                                    edges_sb[:, 0:1], None, ALU.mult)
            nc.vector.tensor_scalar(y1e[:, :, 257:258], y1e[:, :, 257:258],
                                    edges_sb[:, 1:2], None, ALU.mult)
            y2c = act1.tile([P, 4, TP], BF16, tag="h1")
            conv_block(y1e, 3, c2_tiles, TP, bn2s_sb, bn2t_sb, y2c)
            # fc1 (512->256) + gelu
            fc1m = preload(nc.scalar, fc1_wt.ap(), 2, 4, headp, "fc1m",
                           wdt=BF16)
            fc2m = preload(nc.scalar, fc2_wt.ap(), 11, 2, headp, "fc2m",
                           wdt=BF16)
            hfc = act1.tile([P, 2, TP], BF16, tag="qt")
            for mo in range(2):
                ps = ps_mm.tile([P, 512], F32, tag="ps")
                for ko in range(4):
                    nc.tensor.matmul(ps[:, 0:TP], fc1m[:, mo, ko, :],
                                     y2c[:, ko, :],
                                     start=(ko == 0), stop=(ko == 3))
                nc.scalar.activation(hfc[:, mo, :], ps[:, 0:TP], AF.Gelu,
                                     bias=fc1b_sb[:, mo:mo + 1])
            # fc2 (256->1408 padded)
            ologit = act1.tile([P, 11, TP], F32, tag="r1")
            for mo in range(11):
                ps = ps_mm.tile([P, 512], F32, tag="ps")
                for ko in range(2):
                    nc.tensor.matmul(ps[:, 0:TP], fc2m[:, mo, ko, :],
                                     hfc[:, ko, :],
                                     start=(ko == 0), stop=(ko == 1))
                nc.scalar.activation(ologit[:, mo, :], ps[:, 0:TP],
                                     AF.Identity, bias=fc2b_sb[:, mo:mo + 1])
            for mo in range(11):
                nc.sync.dma_start(out_d[mo * P:(mo + 1) * P, :],
                                  ologit[:, mo, :])

    nc.compile()
    return nc


def _prep_inputs(inputs):
    """Host-side: transposes, padding, LN-affine folding, fp8 weight
    scaling, per-core shards."""
    f = lambda k: np.asarray(inputs[k], dtype=np.float32)
    bf = ml_dtypes.bfloat16
    f8 = ml_dtypes.float8_e4m3
    poses = f('poses')
    embed_w, embed_b = f('embed_w'), f('embed_b')
    ln0_g, ln0_b = f('ln0_g'), f('ln0_b')
    inw, inb = f('inw'), f('inb')
    outw, outb = f('outw'), f('outb')
    ln1g, ln1b = f('ln1g'), f('ln1b')
    ln2g, ln2b = f('ln2g'), f('ln2b')
    ff1w, ff1b = f('ff1w'), f('ff1b')
    ff2w, ff2b = f('ff2w'), f('ff2b')
    conv1w, conv1b = f('conv1w'), f('conv1b')
    bn1g, bn1b, bn1m, bn1v = f('bn1g'), f('bn1b'), f('bn1m'), f('bn1v')
    conv2w, conv2b = f('conv2w'), f('conv2b')
    bn2g, bn2b, bn2m, bn2v = f('bn2g'), f('bn2b'), f('bn2m'), f('bn2v')
    fc1w, fc1b = f('fc1w'), f('fc1b')
    fc2w, fc2b = f('fc2w'), f('fc2b')

    def moblk(w_t, nk, nm):
        # [nk*128, nm*128] -> [nm, 128, nk*128]: per-partition contiguous
        return np.ascontiguousarray(
            w_t.reshape(nk, P, nm, P).transpose(2, 1, 0, 3).reshape(nm, P, nk * P))

    def dbl8(ws):
        # fp8 double-quant at one scale: (hi, lo) with hi+lo ~= ws
        hi = ws.astype(f8)
        lo = (ws - hi.astype(np.float32)).astype(f8)
        return hi, lo

    def moblk_dbl(w_t_scaled, nk, nm):
        # [nm, 128, 2, nk*128] fp8 (hi, lo interleaved per mo tile)
        blk = moblk(w_t_scaled, nk, nm)
        hi, lo = dbl8(blk)
        return np.ascontiguousarray(
            np.stack([hi, lo], axis=2).reshape(nm, P, 2 * nk * P))

    shared = {}
    ewt = np.zeros((KIN, D), np.float32)
    ewt[:IN_DIM] = embed_w.T
    shared['emb_wt'] = moblk(ewt, 2, 4)
    shared['emb_b'] = embed_b
    shared['ln0_g'] = ln0_g

    qkv_wt = np.empty((NL, D, 3 * D), np.float32)
    qk_bf = np.empty((NL, 2 * D), np.float32)
    out_bf = np.empty((NL, D), np.float32)
    ff1_wtf = np.empty((NL, D, DFF), np.float32)
    ff1_bf = np.empty((NL, DFF), np.float32)
    ff2_wtf = np.empty((NL, DFF, D), np.float32)
    ff2_bf = np.empty((NL, D), np.float32)
    out_wtf = np.empty((NL, D, D), np.float32)
    for l in range(NL):
        w = inw[l]                      # [3D, D]
        qkv_wt[l] = (w * ln1g[l][None, :]).T
        qkv_bias = inb[l] + w @ ln1b[l]
        qk_bf[l] = qkv_bias[:2 * D] * SW
        out_wtf[l] = outw[l].T * SB
        out_bf[l] = outb[l] + outw[l] @ qkv_bias[2 * D:]
        ff1_wtf[l] = (ff1w[l] * ln2g[l][None, :]).T * SW
        ff1_bf[l] = (ff1b[l] + ff1w[l] @ ln2b[l]) * SW
        ff2_wtf[l] = ff2w[l].T * SB
        ff2_bf[l] = ff2b[l]
    # merged per-layer bias vector: [qk(8*128) | out(4*128) | ff1(16*128) |
    # ff2(4*128)] = 32*128 floats, loaded as one [128, 32] column tile
    shared['lbias'] = np.ascontiguousarray(
        np.concatenate([qk_bf, out_bf, ff1_bf, ff2_bf], axis=1))
    shared['q_wt'] = np.stack(
        [moblk(qkv_wt[l][:, 0:D] * SW, 4, 4) for l in range(NL)]).astype(f8)
    shared['k_wt'] = np.stack(
        [moblk(qkv_wt[l][:, D:2 * D] * SW, 4, 4) for l in range(NL)]).astype(f8)
    # V weights in rhs layout [NL, 2(hi/lo), ko, 128, 512]
    v_s = (qkv_wt[:, :, 2 * D:] * SW).reshape(NL, 4, P, D)
    v_hi, v_lo = dbl8(v_s)
    shared['v_wt'] = np.ascontiguousarray(
        np.stack([v_hi, v_lo], axis=1))
    shared['out_wt'] = np.stack(
        [moblk_dbl(out_wtf[l], 4, 4) for l in range(NL)])
    shared['ff1_wt'] = np.stack(
        [moblk_dbl(ff1_wtf[l], 4, 16) for l in range(NL)])
    shared['ff2_wt'] = np.stack(
        [moblk_dbl(ff2_wtf[l], 16, 4) for l in range(NL)])

    bn1sc = bn1g / np.sqrt(bn1v + EPS)
    bn2sc = bn2g / np.sqrt(bn2v + EPS)
    c1t = conv1w.transpose(2, 1, 0) * 0.5           # [5, D_in, D_out]
    shared['c1_wt'] = np.stack(
        [moblk(c1t[k], 4, 4) for k in range(5)]).astype(bf)
    shared['bn1_s'] = bn1sc
    shared['bn1_t'] = (conv1b - bn1m) * bn1sc + bn1b
    c2t = conv2w.transpose(2, 1, 0)
    shared['c2_wt'] = np.stack(
        [moblk(c2t[k], 4, 4) for k in range(3)]).astype(bf)
    shared['bn2_s'] = bn2sc
    shared['bn2_t'] = (conv2b - bn2m) * bn2sc + bn2b
    shared['fc1_wt'] = moblk(np.ascontiguousarray(fc1w.T), 4, 2).astype(bf)
    shared['fc1_b'] = fc1b
    f2 = np.zeros((D // 2, NCP), np.float32)
    f2[:, :NCLS] = fc2w.T
    shared['fc2_wt'] = moblk(f2, 2, 11).astype(bf)
    f2b = np.zeros((NCP,), np.float32)
    f2b[:NCLS] = fc2b
    shared['fc2_b'] = f2b

    inv = 1.0 / (10000.0 ** (np.arange(0, D, 2, dtype=np.float32) / D))
    si = np.arange(T, dtype=np.float32)[:, None] * inv[None, :]
    pos = np.stack([np.sin(si), np.cos(si)], -1).reshape(T, D)
    pos = pos.astype(np.float32)
    pos_t_g = (pos + ln0_b[None, :]).T.copy()       # [D, T]

    in_maps = []
    for c in range(8):
        b, h = c // 2, c % 2
        own = slice(h * TH, (h + 1) * TH)
        pt = np.zeros((KIN, TH), np.float32)
        pt[:IN_DIM] = poses[b, own].T
        edges_a = np.zeros((P, 2), np.float32)
        edges_a[:, 0] = 1.0 if h == 1 else 0.0
        edges_a[:, 1] = 1.0 if h == 0 else 0.0
        m = dict(shared)
        m['poses_t'] = pt
        m['pos_t'] = pos_t_g[:, own]
        m['edges'] = edges_a
        in_maps.append({k: np.ascontiguousarray(v) for k, v in m.items()})
    return in_maps


def _get_runner():
    """Build the module once and cache a jitted SPMD executable whose weight
    operands stay device-resident between calls."""
    if 'runner' in _CACHE:
        return _CACHE['runner']
    import jax
    import concourse.mybir as mybir_
    from concourse import bass2jax
    from jax.experimental.shard_map import shard_map
    from jax.sharding import Mesh, NamedSharding, PartitionSpec

    nc = _build()
    bass2jax.install_neuronx_cc_hook()
    partition_name = (nc.partition_id_tensor.name
                      if nc.partition_id_tensor else None)
    in_names, out_names, out_avals, zero_outs = [], [], [], []
    for alloc in nc.m.functions[0].allocations:
        if not isinstance(alloc, mybir_.MemoryLocationSet):
            continue
        name = alloc.memorylocations[0].name
        if alloc.kind == "ExternalInput":
            if name != partition_name:
                in_names.append(name)
        elif alloc.kind == "ExternalOutput":
            shape = tuple(alloc.tensor_shape)
            dtype = mybir_.dt.np(alloc.dtype)
            out_names.append(name)
            out_avals.append(jax.core.ShapedArray(shape, dtype))
            zero_outs.append((shape, dtype))
    n_params = len(in_names)
    all_names = in_names + out_names
    if partition_name is not None:
        all_names.append(partition_name)
    donate = tuple(range(n_params, n_params + len(out_names)))

    def _body(*args):
        operands = list(args)
        if partition_name is not None:
            operands.append(bass2jax.partition_id_tensor())
        outs = bass2jax._bass_exec_p.bind(
            *operands,
            out_avals=tuple(out_avals),
            in_names=tuple(all_names),
            out_names=tuple(out_names),
            lowering_input_output_aliases=(),
            sim_require_finite=True,
            sim_require_nnan=True,
            nc=nc,
        )
        return tuple(outs)

    devices = jax.devices()[:8]
    mesh = Mesh(np.asarray(devices), ("core",))
    spec = PartitionSpec("core")
    sharding = NamedSharding(mesh, spec)
    jitted = jax.jit(
        shard_map(_body, mesh=mesh, in_specs=(spec,) * (n_params + len(out_names)),
                  out_specs=(spec,) * len(out_names), check_rep=False),
        donate_argnums=donate, keep_unused=True)

    runner = dict(jitted=jitted, in_names=in_names, out_names=out_names,
                  zero_outs=zero_outs, sharding=sharding)
    _CACHE['runner'] = runner
    return runner


def _put_args(in_maps):
    import jax
    r = _get_runner()
    args = []
    for name in r['in_names']:
        concat = np.concatenate([in_maps[c][name] for c in range(8)], axis=0)
        args.append(jax.device_put(concat, r['sharding']))
    return args


def _exec(args):
    """Run with device-resident input args; returns per-core result dicts.
    Output (donated) buffers are freshly zero-allocated per call."""
    import jax
    r = _get_runner()
    outs_in = [jax.device_put(np.zeros((8 * s[0],) + s[1:], d), r['sharding'])
               for s, d in r['zero_outs']]
    outs = r['jitted'](*args, *outs_in)
    outs = [np.asarray(o) for o in outs]
    return [{name: outs[i].reshape(8, *r['zero_outs'][i][0])[c]
             for i, name in enumerate(r['out_names'])}
            for c in range(8)]


def _run(in_maps):
    return _exec(_put_args(in_maps))


def kernel(**inputs):
    in_maps = _prep_inputs(inputs)
    results = _run(in_maps)
    out = np.empty((B, T // 2, NCLS), np.float32)
    for c in range(8):
        b, h = c // 2, c % 2
        out[b, h * TP:(h + 1) * TP, :] = results[c]['out'][:NCLS].T
    return out


# revision 60
# speedup vs baseline: 1.0018x; 1.0018x over previous
"""Trainium2 Bass kernel for nn_CSLRTransformer (dense transformer, 8 cores).

Sharding: 4 batch elements x 2-way sequence split = 8 cores.
Core c handles batch b=c//2, half h=c%2 (tokens h*512..h*512+511).

K/V rank-order gather: each core computes K,V for its OWN 512 tokens only,
pair-AllGathers them (fp8), then loads BOTH slots back so k_t/v_ext hold
the full 1024-token K/V in global rank order on every core (attention is
key-permutation invariant). The residual stream x lives own-half only;
the conv head needs just a 12-token edge halo, gathered once after layer 7.

Numerics: the transformer body runs fp8e4m3 matmuls in DoubleRow perf mode
(2 stationary rows per PE cell, 0.5 cycles/moving-row = 4x bf16):
  - weight linears (QKV/out/FFN) pair adjacent 128-blocks of the
    contraction: lhsT [128,2,128], rhs [128,2,N].
  - scores contract dh=64 per head; the pair dim is (real block, zero
    block): q_t/k_t carry a 5th all-zero mo-block and the AP stride trick
    q_t[h*64:, mo:5:(4-mo), :] yields [64,2,N] pairs (real, zero).
  - context pairs adjacent 128-token blocks of the key axis.
Weight tensors are pre-scaled on the host into fp8's normal range and the
scales fold away for free: Q/K scales (64x each) fold into the softmax exp
scale; the V scale cancels in the softmax division (the denominator ones
columns in v_ext are memset to the same 64.0); FFN1's scale commutes with
ReLU (r1 is stored scaled, FFN2's evac descales); out/FFN2 descale via the
evac's scale operand. Residual stream and LN stats stay f32; matmuls
accumulate in f32 PSUM. The conv/fc head stays bf16 (cheap, and its error
does not average out).

Softmax: exp on ACT (the dominant ACT load); denominators ride as 64
replicated ones-columns on v_ext so the context matmul lands them on PSUM
partitions 64:127 -- reciprocal (DVE) + multiply (Pool) finish the
normalization without the old PE broadcast matmul. Linear evacuations are
spread across DVE/Pool via tensor_scalar (scale+bias in one op) to keep
ACT free for exp; LN keeps ln/exp (rstd = exp(-0.5*ln(var))) on ACT with
the stats transforms moved to DVE/Pool. The get_activation_tables patch
steers every used ACT function into the natural_log_exp_and_others set so
the whole transformer body runs on one function table.
"""
import numpy as np
import ml_dtypes

import concourse.bacc as bacc
import concourse.bass as bass
import concourse.mybir as mybir
import concourse.tile as tile
from concourse.bass_utils import run_bass_kernel_spmd

# Steer bass's greedy ACT-table assignment (first set containing the
# function) so ln/exp/identity all pick natural_log_exp_and_others: remove
# them from the sets that precede it in act_info.json order. Only the
# ASSIGNMENT view changes -- emitted act_func_set_ids still index the real
# act_info.json, and the chosen set genuinely contains these functions, so
# hardware behavior is correct; it just stops reloading tables every chain.
_real_get_act_tables = None


def _patched_get_act_tables(arch):
    tabs = dict(_real_get_act_tables(arch))
    target = "natural_log_exp_and_others"
    if target in tabs:
        steer = {a for a in tabs[target]
                 if a.name.lower() in ("ln", "exp", "identity", "square",
                                       "copy", "relu")}
        seen_target = False
        out = {}
        for name, s in tabs.items():
            if name == target:
                seen_target = True
            out[name] = s if seen_target else (s - steer)
        return out
    return tabs


def _install_act_table_patch():
    global _real_get_act_tables
    if _real_get_act_tables is None:
        _real_get_act_tables = bacc.get_activation_tables
        bacc.get_activation_tables = _patched_get_act_tables


_install_act_table_patch()

dt = mybir.dt
AF = mybir.ActivationFunctionType
ALU = mybir.AluOpType
DRow = mybir.MatmulPerfMode.DoubleRow

P = 128
B, T, IN_DIM, D, H, NCLS = 4, 1024, 231, 512, 8, 1296
NL, DFF, DH = 8, 2048, 64
TH = T // 2            # 512 own tokens
TP = TH // 2           # 256 own pooled positions
KIN = 256              # padded embed contraction (231 -> 256)
NCP = 1408             # padded classes (1296 -> 11*128)
EPS = 1e-5
F32 = dt.float32
F32R = dt.float32r
BF16 = dt.bfloat16
F8 = dt.float8e4

SW = 64.0              # scale for q/k/v/ff1 weights (activations stay scaled)
SB = 256.0             # scale for out/ff2 weights (descaled at evac)
EXP_SCALE = 0.125 / (SW * SW)
OUT_SCALE = 1.0 / SB
FF2_SCALE = 1.0 / (SW * SB)

_CACHE = {}


def _build(single_core=False):
    nc = bacc.Bacc("TRN2", target_bir_lowering=False, debug=False, num_devices=8)

    # ---- DRAM I/O ----
    poses_t = nc.dram_tensor("poses_t", [KIN, TH], F32R, kind="ExternalInput")
    pos_t = nc.dram_tensor("pos_t", [D, TH], F32, kind="ExternalInput")
    edges = nc.dram_tensor("edges", [P, 2], F32, kind="ExternalInput")
    emb_wt = nc.dram_tensor("emb_wt", [4, P, KIN], F32R, kind="ExternalInput")
    emb_b = nc.dram_tensor("emb_b", [D], F32, kind="ExternalInput")
    ln0_g = nc.dram_tensor("ln0_g", [D], F32, kind="ExternalInput")
    q_wt = nc.dram_tensor("q_wt", [NL, 4, P, D], F8, kind="ExternalInput")
    k_wt = nc.dram_tensor("k_wt", [NL, 4, P, D], F8, kind="ExternalInput")
    # double-quantized (hi, lo) fp8 weights: lo = fp8(w*scale - hi), both at
    # the same scale so they accumulate into one PSUM with one descale
    v_wt = nc.dram_tensor("v_wt", [NL, 2, 4, P, D], F8, kind="ExternalInput")
    out_wt = nc.dram_tensor("out_wt", [NL, 4, P, 2 * D], F8,
                            kind="ExternalInput")
    ff1_wt = nc.dram_tensor("ff1_wt", [NL, 16, P, 2 * D], F8,
                            kind="ExternalInput")
    ff2_wt = nc.dram_tensor("ff2_wt", [NL, 4, P, 2 * DFF], F8,
                            kind="ExternalInput")
    lbias = nc.dram_tensor("lbias", [NL, 32 * P], F32, kind="ExternalInput")
    c1_wt = nc.dram_tensor("c1_wt", [5, 4, P, D], BF16, kind="ExternalInput")
    bn1_s = nc.dram_tensor("bn1_s", [D], F32, kind="ExternalInput")
    bn1_t = nc.dram_tensor("bn1_t", [D], F32, kind="ExternalInput")
    c2_wt = nc.dram_tensor("c2_wt", [3, 4, P, D], BF16, kind="ExternalInput")
    bn2_s = nc.dram_tensor("bn2_s", [D], F32, kind="ExternalInput")
    bn2_t = nc.dram_tensor("bn2_t", [D], F32, kind="ExternalInput")
    fc1_wt = nc.dram_tensor("fc1_wt", [2, P, D], BF16, kind="ExternalInput")
    fc1_b = nc.dram_tensor("fc1_b", [D // 2], F32, kind="ExternalInput")
    fc2_wt = nc.dram_tensor("fc2_wt", [11, P, D // 2], BF16, kind="ExternalInput")
    fc2_b = nc.dram_tensor("fc2_b", [NCP], F32, kind="ExternalInput")
    out_d = nc.dram_tensor("out", [NCP, TP], F32, kind="ExternalOutput")

    with tile.TileContext(nc) as tc:
        with (
            tc.tile_pool(name="state", bufs=1) as state,
            tc.tile_pool(name="act1", bufs=1) as act1,
            tc.tile_pool(name="act2", bufs=3) as act2,
            tc.tile_pool(name="wts", bufs=3) as wts,
            tc.tile_pool(name="wvp", bufs=2) as wvp,
            tc.tile_pool(name="kqp", bufs=2) as kqp,
            tc.tile_pool(name="ff1p", bufs=1) as ff1p,
            tc.tile_pool(name="ff2p", bufs=1) as ff2p,
            tc.tile_pool(name="biasp", bufs=2) as biasp,
            tc.tile_pool(name="headp", bufs=1) as headp,
            tc.tile_pool(name="ps_mm", bufs=2, space="PSUM") as ps_mm,
            tc.tile_pool(name="ps_ctx", bufs=2, space="PSUM") as ps_ctx,
            tc.tile_pool(name="ps_sc", bufs=2, space="PSUM") as ps_sc,
            tc.tile_pool(name="dram", bufs=2, space="DRAM") as dram,
        ):
            # ---------- constants / persistent ----------
            ones_f32 = state.tile([P, P], F32)
            nc.vector.memset(ones_f32[:], 1.0)
            ones_sq = state.tile([P, P], F32R)
            nc.vector.tensor_copy(ones_sq[:], ones_f32[:])
            ones_bf = state.tile([P, P], BF16)
            nc.vector.tensor_copy(ones_bf[:], ones_f32[:])
            x_sb = state.tile([P, 4, TH], F32R)      # residual stream (own)
            xblk = state.tile([P, 4, TH], F32R)      # block-residual save
            # [ktok, kt, head, dh | 64 ones(=SW) denominator columns]: the
            # ones value matches the V weight scale so the softmax division
            # cancels it exactly
            v_ext = state.tile([P, 8, H, 128], F8)
            nc.vector.memset(v_ext[:, :, :, 64:128], SW)
            # q_t/k_t mo-blocks 0-3 hold data; block 4 is the all-zero
            # DoubleRow pair for the 64-deep score contraction
            q_t = state.tile([P, 5, TH], F8)
            nc.vector.memset(q_t[:, 4, :], 0.0)
            k_t = state.tile([P, 5, T], F8)
            nc.vector.memset(k_t[:, 4, :], 0.0)
            epsb = state.tile([P, 1], F32)
            nc.vector.memset(epsb[:], EPS)

            def load_pcol(dr, n, eng=None):
                # [n*128] dram vector -> [128, n] sbuf (d on partitions)
                t_ = state.tile([P, n], F32, tag=f"b{n}_{dr.tensor.name}")
                (eng or nc.sync).dma_start(t_[:],
                                           dr.rearrange("(o p) -> p o", p=P))
                return t_

            emb_b_sb = load_pcol(emb_b.ap(), 4)
            ln0g_sb = load_pcol(ln0_g.ap(), 4)
            bn1s_sb = load_pcol(bn1_s.ap(), 4)
            bn1t_sb = load_pcol(bn1_t.ap(), 4)
            bn2s_sb = load_pcol(bn2_s.ap(), 4)
            bn2t_sb = load_pcol(bn2_t.ap(), 4)
            fc1b_sb = load_pcol(fc1_b.ap(), 2)
            fc2b_sb = load_pcol(fc2_b.ap(), 11)
            edges_sb = state.tile([P, 2], F32)
            nc.sync.dma_start(edges_sb[:], edges[:])

            # ---------- helpers ----------
            def linear(x, w_dram, nk, nm, ncols, evac, out, wdt=F8,
                       wtiles=None, dr=True, dbl=False):
                """out[:, mo, :ncols] = w.T @ x (DoubleRow fp8 by default).
                w_dram: mo-blocked AP [nm, 128, nk*128] (or [..., 2*nk*128]
                hi/lo when dbl); wtiles: preloaded 3D weight tiles."""
                pp = [None, 0]
                nrep = 2 if dbl else 1
                for mo in range(nm):
                    if wtiles is not None:
                        wt = wtiles[:, mo]
                    else:
                        shape = [P, 2, nk, P] if dbl else [P, nk, P]
                        wt = wts.tile(shape, wdt, tag="wmo")
                        eng = nc.gpsimd if mo % 2 == 0 else nc.sync
                        eng.dma_start(wt[:], w_dram[mo])
                    for nti in range((ncols + 511) // 512):
                        cs = min(512, ncols - nti * 512)
                        if pp[0] is None or pp[1] == 1:
                            ppt = ps_sc.tile([P, 2, 512], F32, tag="psp")
                            pp[0] = ppt
                            pp[1] = 0
                        ps = pp[0][:, pp[1], :]
                        pp[1] += 1
                        if dr:
                            for rep in range(nrep):
                                wr = wt[:, rep] if dbl else wt
                                for kp in range(nk // 2):
                                    nc.tensor.matmul(
                                        ps[:, :cs],
                                        wr[:, 2 * kp:2 * kp + 2, :],
                                        x[:, 2 * kp:2 * kp + 2,
                                          nti * 512:nti * 512 + cs],
                                        start=(rep == 0 and kp == 0),
                                        stop=(rep == nrep - 1
                                              and kp == nk // 2 - 1),
                                        perf_mode=DRow)
                        else:
                            for ko in range(nk):
                                nc.tensor.matmul(
                                    ps[:, :cs],
                                    wt[:, ko, :],
                                    x[:, ko, nti * 512:nti * 512 + cs],
                                    start=(ko == 0), stop=(ko == nk - 1))
                        evac(ps[:, :cs], mo, nti * 512, out)
                return out

            def warm_pre(n):
                # dependency-free matmuls that keep the PE p-state hot while
                # the preceding elementwise chain (residual adds etc.) runs
                wps = ps_mm.tile([P, 512], F32, tag="ps")
                for _ in range(n):
                    nc.tensor.matmul(wps[:, 0:P], ones_bf[:], ones_bf[:],
                                     start=True, stop=True)

            def preload(eng, w_dram, nm, nk, pool, tag, wdt=F8, dbl=False):
                """Whole weight matrix in ONE merged DMA: tile
                [P, nm, (2,) nk, P] <- dram [nm, P, (2*)nk*P]. Double-
                buffered pools make the DMA wait-free, so it never clogs
                its HWDGE ring."""
                shape = ([P, nm, 2, nk, P] if dbl else [P, nm, nk, P])
                wt = pool.tile(shape, wdt, tag=tag)
                eng.dma_start(wt[:], w_dram.rearrange("m p z -> p m z"))
                return wt

            def evac_bias(bias_sb, func=AF.Identity, scale=1.0):
                def _e(ps, mo, c0, out):
                    nc.scalar.activation(
                        out[:, mo, c0:c0 + ps.shape[-1]], ps,
                        func, bias=bias_sb[:, mo:mo + 1], scale=scale)
                return _e

            def evac_bias_ts(eng, bias_sb, scale=None):
                # (ps * scale + bias) on DVE/Pool in one tensor_scalar
                def _e(ps, mo, c0, out):
                    o = out[:, mo, c0:c0 + ps.shape[-1]]
                    if scale is None:
                        eng.tensor_scalar(o, ps, bias_sb[:, mo:mo + 1],
                                          None, ALU.add)
                    else:
                        eng.tensor_scalar(o, ps, scale,
                                          bias_sb[:, mo:mo + 1],
                                          ALU.mult, ALU.add)
                return _e

            def evac_relu_ts(eng, bias_sb):
                def _e(ps, mo, c0, out):
                    eng.tensor_scalar(
                        out[:, mo, c0:c0 + ps.shape[-1]], ps,
                        bias_sb[:, mo:mo + 1], 0.0, ALU.add, ALU.max)
                return _e

            def evac_mix(*fns):
                def _e(ps, mo, c0, out):
                    fns[mo % len(fns)](ps, mo, c0, out)
                return _e

            def ln_part(x, out, c0, cs, gamma=None, warm=0):
                """LayerNorm over columns [c0, c0+cs): stats via ones-matmul,
                transforms spread over DVE/Pool, rstd = exp(-0.5*ln(var)) on
                ACT (shared softmax table). `warm` extra matmuls re-fill ps1
                after its readers finish, keeping the PE p-state hot through
                the elementwise chain."""
                sl = slice(c0, c0 + cs)
                sq = act1.tile([P, 4, 512], F32R, tag="sq")
                ps1 = ps_mm.tile([P, 512], F32, tag="ps")
                for ko in range(4):
                    nc.tensor.matmul(ps1[:, :cs], ones_sq[:], x[:, ko, sl],
                                     start=(ko == 0), stop=(ko == 3))
                for ko in range(4):
                    nc.vector.tensor_tensor(sq[:, ko, sl], x[:, ko, sl],
                                            x[:, ko, sl], ALU.mult)
                ps2 = ps_mm.tile([P, 512], F32, tag="ps")
                for ko in range(4):
                    nc.tensor.matmul(ps2[:, :cs], ones_sq[:], sq[:, ko, sl],
                                     start=(ko == 0), stop=(ko == 3))
                m2 = act1.tile([P, 512], F32, tag="m2")
                va = act1.tile([P, 512], F32, tag="va")
                r = act1.tile([P, 512], F32, tag="r")
                m = act1.tile([P, 512], F32, tag="m")
                # m2 = (S1/D)^2 on ACT in parallel with m = S1/D on DVE;
                # then ONE DVE stt: va = S2/D - m2 (EPS folds into Ln bias)
                nc.scalar.activation(m2[:, :cs], ps1[:, :cs], AF.Square,
                                     scale=1.0 / D)
                nc.vector.tensor_scalar(m[:, :cs], ps1[:, :cs], 1.0 / D,
                                        None, ALU.mult)
                nc.vector.scalar_tensor_tensor(
                    va[:, :cs], ps2[:, :cs], 1.0 / D, m2[:, :cs],
                    ALU.mult, ALU.subtract)
                # rstd = exp(-0.5*ln(var + EPS))
                nc.scalar.activation(va[:, :cs], va[:, :cs], AF.Ln,
                                     bias=epsb[:, 0:1])
                nc.scalar.activation(r[:, :cs], va[:, :cs], AF.Exp,
                                     scale=-0.5)
                for i in range(warm):
                    nc.tensor.matmul(ps1[:, :cs], ones_sq[:],
                                     x[:, i % 4, sl], start=True, stop=True)
                for ko in range(4):
                    eng = nc.vector if ko % 2 == 0 else nc.gpsimd
                    eng.tensor_tensor(sq[:, ko, sl], x[:, ko, sl],
                                      m[:, :cs], ALU.subtract)
                    eng.tensor_tensor(out[:, ko, sl], sq[:, ko, sl],
                                      r[:, :cs], ALU.mult)
                    if gamma is not None:
                        eng.tensor_scalar(
                            out[:, ko, sl], out[:, ko, sl],
                            gamma[:, ko:ko + 1], None, ALU.mult)

            def ln_chain(x, out, gamma=None):
                ln_part(x, out, 0, 512, gamma=gamma)

            def linear_chunk(x, wtiles, nk, nm, c0, cs, evac, out, dbl=False,
                             mos=None):
                """Column-range piece of a linear from preloaded weights."""
                pp = [None, 0]
                nrep = 2 if dbl else 1
                for mo in (mos if mos is not None else range(nm)):
                    if pp[0] is None or pp[1] == 1:
                        ppt = ps_sc.tile([P, 2, 512], F32, tag="psp")
                        pp[0] = ppt
                        pp[1] = 0
                    ps = pp[0][:, pp[1], :]
                    pp[1] += 1
                    for rep in range(nrep):
                        wr = wtiles[:, mo, rep] if dbl else wtiles[:, mo]
                        for kp in range(nk // 2):
                            nc.tensor.matmul(
                                ps[:, :cs], wr[:, 2 * kp:2 * kp + 2, :],
                                x[:, 2 * kp:2 * kp + 2, c0:c0 + cs],
                                start=(rep == 0 and kp == 0),
                                stop=(rep == nrep - 1 and kp == nk // 2 - 1),
                                perf_mode=DRow)
                    evac(ps[:, :cs], mo, c0, out)

            # ---------- embed + LN0 + pos ----------
            poses_sb = act1.tile([P, 2, TH], F32R, tag="qt")
            for ko in range(2):
                nc.gpsimd.dma_start(poses_sb[:, ko, :],
                                    poses_t[ko * P:(ko + 1) * P, :])
            xe = act1.tile([P, 4, TH], F32R, tag="r1")
            linear(poses_sb, emb_wt.ap(), 2, 4, TH, evac_bias(emb_b_sb), xe,
                   wdt=F32R, dr=False)
            ln_chain(xe, x_sb, gamma=ln0g_sb)
            for ko in range(4):
                nc.gpsimd.dma_start(xblk[:, ko, :],
                                    pos_t[ko * P:(ko + 1) * P, :])
            for ko in range(4):
                nc.vector.tensor_tensor(x_sb[:, ko, :], x_sb[:, ko, :],
                                        xblk[:, ko, :], ALU.add)

            # ---------- transformer layers ----------
            def prefetch_qkv(li):
                # merged single-DMA weight prefetch on the scalar/vector
                # rings; the sync ring is reserved for the gather chain
                kw = preload(nc.scalar, k_wt[li], 4, 4, kqp, "kw")
                qw = preload(nc.scalar, q_wt[li], 4, 4, kqp, "qw")
                wv = wvp.tile([P, 2, 4, 512], F8, tag="wv")
                nc.scalar.dma_start(
                    wv[:], v_wt[li].rearrange("r k p d -> p r k d"))
                return kw, qw, wv

            nxt = prefetch_qkv(0)
            for li in range(NL):
                kw_t, qw_t, wv = nxt
                # per-layer biases: ONE merged DMA -> [128, 32] column tile
                lb = biasp.tile([P, 32], F32, tag="lb")
                nc.scalar.dma_start(lb[:],
                                    lbias[li].rearrange("(o p) -> p o", p=P))
                qkb_sb = lb[:, 0:8]
                outb_sb = lb[:, 8:12]
                ff1b_sb = lb[:, 12:28]
                ff2b_sb = lb[:, 28:32]
                # FFN weights for this layer (first use ~15us in)
                ff1w_t = preload(nc.scalar, ff1_wt[li], 16, 4, ff1p, "ff1w",
                                 dbl=True)
                ff2w_t = preload(nc.scalar, ff2_wt[li], 4, 16, ff2p, "ff2w",
                                 dbl=True)
                outw_t = preload(nc.scalar, out_wt[li], 4, 4, ff2p, "outw",
                                 dbl=True)

                # LN1 -> K (K matmuls chase the LN chain); K is emitted
                # mo-group-first (mo 0/1 over both column chunks, then 2/3)
                # so the first gather launches as early as possible
                warm_pre(6)
                h1 = act1.tile([P, 4, TH], F8, tag="h1")
                b_ik1 = dram.tile([2 * P, TH], F8, tag="kin1")
                b_ok1 = dram.tile([2, 2 * P, TH], F8, tag="kout1")
                b_ik2 = dram.tile([2 * P, TH], F8, tag="kin2")
                b_ok2 = dram.tile([2, 2 * P, TH], F8, tag="kout2")
                evk = evac_mix(evac_bias_ts(nc.vector, qkb_sb[:, 4:]),
                               evac_bias(qkb_sb[:, 4:]))

                def k_gather(b_ik_, b_ok_, koff):
                    # entire chain on the sync HWDGE ring, in order
                    bik_ = b_ik_.rearrange("(ko p) t -> p ko t", p=P)
                    nc.sync.dma_start(bik_[:, 0:2, :],
                                      k_t[:, koff:koff + 2, 0:TH])
                    if single_core:
                        nc.sync.dma_start(b_ok_[0], b_ik_[:])
                        nc.sync.dma_start(b_ok_[1], b_ik_[:])
                    else:
                        nc.gpsimd.collective_compute(
                            "AllGather", ALU.bypass,
                            ins=[b_ik_.opt()], outs=[b_ok_.opt()],
                            replica_groups=[[0, 1], [2, 3], [4, 5], [6, 7]])
                    for slot in range(2):
                        eng = nc.sync if slot == 0 else nc.scalar
                        eng.dma_start(
                            k_t[:, koff:koff + 2, slot * TH:(slot + 1) * TH],
                            b_ok_[slot].rearrange("(ko p) t -> p ko t", p=P))

                ln_part(x_sb, h1, 0, 256, warm=6)
                linear_chunk(h1, kw_t, 4, 4, 0, 256, evk, k_t, mos=(0, 1))
                ln_part(x_sb, h1, 256, 256)
                linear_chunk(h1, kw_t, 4, 4, 256, 256, evk, k_t, mos=(0, 1))
                k_gather(b_ik1, b_ok1, 0)
                linear_chunk(h1, kw_t, 4, 4, 0, 256, evk, k_t, mos=(2, 3))
                linear_chunk(h1, kw_t, 4, 4, 256, 256, evk, k_t, mos=(2, 3))
                k_gather(b_ik2, b_ok2, 2)

                # Q own (feeds scores as soon as the gather lands)
                evq = evac_mix(evac_bias_ts(nc.vector, qkb_sb[:, 0:]),
                               evac_bias(qkb_sb[:, 0:]))
                linear(h1, None, 4, 4, TH, evq, q_t, wtiles=qw_t)

                # V own (token-major via stationary-activation trick)
                v_stg = act1.tile([P, 4, H, DH], F8, tag="vstg")
                for tt in range(4):
                    ps = ps_mm.tile([P, 512], F32, tag="ps")
                    for rep in range(2):
                        for kp in range(2):
                            nc.tensor.matmul(
                                ps[:],
                                h1[:, 2 * kp:2 * kp + 2, tt * P:(tt + 1) * P],
                                wv[:, rep, 2 * kp:2 * kp + 2, :],
                                start=(rep == 0 and kp == 0),
                                stop=(rep == 1 and kp == 1),
                                perf_mode=DRow)
                    nc.vector.tensor_copy(
                        v_stg[:, tt],
                        ps[:].rearrange("p (h d) -> p h d", d=DH))
                b_iv = dram.tile([TH, D], F8, tag="vin")
                b_ov = dram.tile([2, TH, D], F8, tag="vout")
                nc.sync.dma_start(
                    b_iv.rearrange("(tt p) (h d) -> p tt h d", p=P, d=DH),
                    v_stg[:])
                if single_core:
                    nc.sync.dma_start(b_ov[0], b_iv[:])
                    nc.sync.dma_start(b_ov[1], b_iv[:])
                else:
                    nc.gpsimd.collective_compute(
                        "AllGather", ALU.bypass,
                        ins=[b_iv.opt()], outs=[b_ov.opt()],
                        replica_groups=[[0, 1], [2, 3], [4, 5], [6, 7]])
                for slot in range(2):
                    bovr = b_ov[slot].rearrange("(tt p) (h d) -> p tt h d",
                                                p=P, d=DH)
                    for tt in range(4):
                        # split across both HWDGE rings (scalar ring is idle
                        # during attention)
                        eng = nc.sync if tt % 2 == 0 else nc.scalar
                        eng.dma_start(
                            v_ext[:, slot * 4 + tt, :, 0:DH], bovr[:, tt])

                # pre-add the out-proj bias to the residual stream now (x is
                # idle through attention); the out evac then fuses the
                # residual add via scalar_tensor_tensor
                for ko in range(4):
                    nc.gpsimd.tensor_scalar(x_sb[:, ko, :], x_sb[:, ko, :],
                                            outb_sb[:, ko:ko + 1], None,
                                            ALU.add)

                if li == NL - 1:
                    c1_tiles = [preload(nc.scalar, c1_wt[k], 4, 4, headp,
                                        f"c1_{k}", wdt=BF16)
                                for k in range(5)]
                    c2_tiles = [preload(nc.scalar, c2_wt[k], 4, 4, headp,
                                        f"c2_{k}", wdt=BF16)
                                for k in range(3)]

                # attention, kt-phased: slot-0 scores for every head pair
                # first, so slot-1's gather latency hides behind them.
                # scores: DoubleRow with the (real, zero-block) pair trick;
                # context: DoubleRow over adjacent key-token blocks, with the
                # denominator riding on psum partitions 64:127 via the 64
                # ones-columns of v_ext.
                ctx = act1.tile([P, 4, TH], F8, tag="ctx")
                for mo in range(4):
                    p_t = act2.tile([P, 8, 2, TH], F8, tag="pt")
                    st = 4 - mo
                    for kt in range(8):
                        pp = ps_sc.tile([P, 2, 512], F32, tag="psp")
                        for hh in range(2):
                            nc.tensor.matmul(
                                pp[:, hh, :],
                                k_t[hh * DH:(hh + 1) * DH, mo:5:st,
                                    kt * P:(kt + 1) * P],
                                q_t[hh * DH:(hh + 1) * DH, mo:5:st, :],
                                start=True, stop=True, perf_mode=DRow)
                        nc.scalar.activation(p_t[:, kt], pp[:], AF.Exp,
                                             scale=EXP_SCALE)
                    for hh in range(2):
                        h = 2 * mo + hh
                        bp = hh * 64
                        psc = ps_ctx.tile([P, 512], F32, tag="ps_ctx")
                        for tp in range(4):
                            nc.tensor.matmul(
                                psc[:],
                                v_ext[:, 2 * tp:2 * tp + 2, h, :],
                                p_t[:, 2 * tp:2 * tp + 2, hh, :],
                                start=(tp == 0), stop=(tp == 3),
                                perf_mode=DRow)
                        rcp = act1.tile([64, TH], F32R, tag="rcp")
                        with nc.allow_low_precision(reason="softmax denom"):
                            nc.vector.reciprocal(rcp[:], psc[64:128, :])
                        nc.vector.tensor_tensor(
                            ctx[bp:bp + 64, mo, :], psc[0:64, :],
                            rcp[:], ALU.mult)

                # prefetch next layer's K/Q/V weights while attention runs
                if li + 1 < NL:
                    nxt = prefetch_qkv(li + 1)

                # out-proj: evac fuses the residual add (bias was pre-added
                # to x during attention): x += psum * OUT_SCALE, one DVE op
                def evac_resid(scale):
                    def _e(ps, mo, c0, out):
                        nc.vector.scalar_tensor_tensor(
                            out[:, mo, c0:c0 + ps.shape[-1]], ps, scale,
                            out[:, mo, c0:c0 + ps.shape[-1]],
                            ALU.mult, ALU.add)
                    return _e

                linear(ctx, None, 4, 4, TH, evac_resid(OUT_SCALE), x_sb,
                       wtiles=outw_t, dbl=True)

                # FFN; LN2 -> FFN1 in 256-col chunks; ff2 bias pre-added to
                # x during the ff1 phase, ff2 evac fuses the residual add
                h2 = act1.tile([P, 4, TH], F8, tag="h1")
                r1 = act1.tile([P, 16, TH], F8, tag="r1")
                warm_pre(6)
                evf = evac_mix(evac_relu_ts(nc.vector, ff1b_sb),
                               evac_bias(ff1b_sb, func=AF.Relu))
                for c0 in (0, 256):
                    ln_part(x_sb, h2, c0, 256, warm=(6 if c0 == 0 else 0))
                    linear_chunk(h2, ff1w_t, 4, 16, c0, 256, evf, r1,
                                 dbl=True)
                for ko in range(4):
                    nc.gpsimd.tensor_scalar(x_sb[:, ko, :], x_sb[:, ko, :],
                                            ff2b_sb[:, ko:ko + 1], None,
                                            ALU.add)
                linear(r1, None, 16, 4, TH, evac_resid(FF2_SCALE), x_sb,
                       wtiles=ff2w_t, dbl=True)

                # block residual: y = block(y) + y at layers 3, 5, 7
                if li in (3, 5, 7):
                    for ko in range(4):
                        nc.vector.tensor_tensor(x_sb[:, ko, :],
                                                x_sb[:, ko, :],
                                                xblk[:, ko, :], ALU.add)
                if li in (1, 3, 5):
                    for ko in range(4):
                        nc.gpsimd.tensor_copy(xblk[:, ko, :], x_sb[:, ko, :])

            # ---------- conv-edge halo gather (12 raw tokens) ----------
            b_ie = dram.tile([D, 12], F32R, tag="egin")
            b_oe = dram.tile([2, D, 12], F32R, tag="egout")
            bie = b_ie.rearrange("(ko p) t -> p ko t", p=P)
            nc.sync.dma_start(bie[:, :, 0:6], x_sb[:, :, 0:6])
            nc.sync.dma_start(bie[:, :, 6:12], x_sb[:, :, 506:512])
            if single_core:
                nc.gpsimd.dma_start(b_oe[0], b_ie[:])
                nc.sync.dma_start(b_oe[1], b_ie[:])
            else:
                nc.gpsimd.collective_compute(
                    "AllGather", ALU.bypass,
                    ins=[b_ie.opt()], outs=[b_oe.opt()],
                    replica_groups=[[0, 1], [2, 3], [4, 5], [6, 7]])
            # pool the interior while the edge gather is in flight
            xpe = act1.tile([P, 4, 262], BF16, tag="xpe")
            nc.vector.tensor_tensor(xpe[:, :, 3:259], x_sb[:, :, 0:TH:2],
                                    x_sb[:, :, 1:TH:2], ALU.add)
            s0 = act1.tile([P, 4, 12], F32R, tag="s0")
            s1 = act1.tile([P, 4, 12], F32R, tag="s1")
            nc.sync.dma_start(s0[:], b_oe[0].rearrange("(ko p) t -> p ko t", p=P))
            nc.sync.dma_start(s1[:], b_oe[1].rearrange("(ko p) t -> p ko t", p=P))
            pe12 = act1.tile([P, 4, 12], F32R, tag="pe12")
            nc.vector.tensor_tensor(pe12[:], s0[:], s1[:], ALU.add)
            nc.vector.tensor_tensor(pe12[:, :, 0:6], pe12[:, :, 0:6],
                                    x_sb[:, :, 0:6], ALU.subtract)
            nc.vector.tensor_tensor(pe12[:, :, 6:12], pe12[:, :, 6:12],
                                    x_sb[:, :, 506:512], ALU.subtract)
            ph = act1.tile([P, 4, 6], F32, tag="ph")
            nc.vector.tensor_tensor(ph[:], pe12[:, :, 0:12:2],
                                    pe12[:, :, 1:12:2], ALU.add)

            # ---------- head: pool -> conv1 -> conv2 -> fc1 -> fc2 ----------
            # (avg-pool(2) interior already computed above; the 0.5 factor is
            # folded into conv1 weights)
            # halo: ph[3:6]=peer high edge (left halo), ph[0:3]=peer low (right)
            nc.vector.tensor_scalar(xpe[:, :, 0:3], ph[:, :, 3:6],
                                    edges_sb[:, 0:1], None, ALU.mult)
            nc.vector.tensor_scalar(xpe[:, :, 259:262], ph[:, :, 0:3],
                                    edges_sb[:, 1:2], None, ALU.mult)

            def conv_block(src, ntaps, wtiles, ncols, bn_s, bn_t, out):
                # wtiles: per-tap merged [128, 4(mo), 4(ko), 128]
                for mo in range(4):
                    ps = ps_mm.tile([P, 512], F32, tag="ps")
                    for k in range(ntaps):
                        for ko in range(4):
                            nc.tensor.matmul(
                                ps[:, 0:ncols], wtiles[k][:, mo, ko, :],
                                src[:, ko, k:k + ncols],
                                start=(k == 0 and ko == 0),
                                stop=(k == ntaps - 1 and ko == 3))
                    nc.scalar.activation(out[:, mo, :], ps[:, 0:ncols],
                                         AF.Gelu, bias=bn_t[:, mo:mo + 1],
                                         scale=bn_s[:, mo:mo + 1])

            y1e = act1.tile([P, 4, 258], BF16, tag="ysb")
            conv_block(xpe, 5, c1_tiles, 258, bn1s_sb, bn1t_sb, y1e)
            # conv2 zero-pads at the GLOBAL sequence edges: kill the computed
            # y1 halo column on the outer side of each boundary core
            nc.vector.tensor_scalar(y1e[:, :, 0:1], y1e[:, :, 0:1],
                                    edges_sb[:, 0:1], None, ALU.mult)
            nc.vector.tensor_scalar(y1e[:, :, 257:258], y1e[:, :, 257:258],
                                    edges_sb[:, 1:2], None, ALU.mult)
            y2c = act1.tile([P, 4, TP], BF16, tag="h1")
            conv_block(y1e, 3, c2_tiles, TP, bn2s_sb, bn2t_sb, y2c)
            # fc1 (512->256) + gelu
            fc1m = preload(nc.scalar, fc1_wt.ap(), 2, 4, headp, "fc1m",
                           wdt=BF16)
            fc2m = preload(nc.scalar, fc2_wt.ap(), 11, 2, headp, "fc2m",
                           wdt=BF16)
            hfc = act1.tile([P, 2, TP], BF16, tag="qt")
            for mo in range(2):
                ps = ps_mm.tile([P, 512], F32, tag="ps")
                for ko in range(4):
                    nc.tensor.matmul(ps[:, 0:TP], fc1m[:, mo, ko, :],
                                     y2c[:, ko, :],
                                     start=(ko == 0), stop=(ko == 3))
                nc.scalar.activation(hfc[:, mo, :], ps[:, 0:TP], AF.Gelu,
                                     bias=fc1b_sb[:, mo:mo + 1])
            # fc2 (256->1408 padded)
            ologit = act1.tile([P, 11, TP], F32, tag="r1")
            for mo in range(11):
                ps = ps_mm.tile([P, 512], F32, tag="ps")
                for ko in range(2):
                    nc.tensor.matmul(ps[:, 0:TP], fc2m[:, mo, ko, :],
                                     hfc[:, ko, :],
                                     start=(ko == 0), stop=(ko == 1))
                nc.scalar.activation(ologit[:, mo, :], ps[:, 0:TP],
                                     AF.Identity, bias=fc2b_sb[:, mo:mo + 1])
            for mo in range(11):
                nc.sync.dma_start(out_d[mo * P:(mo + 1) * P, :],
                                  ologit[:, mo, :])

    nc.compile()
    return nc


def _prep_inputs(inputs):
    """Host-side: transposes, padding, LN-affine folding, fp8 weight
    scaling, per-core shards."""
    f = lambda k: np.asarray(inputs[k], dtype=np.float32)
    bf = ml_dtypes.bfloat16
    f8 = ml_dtypes.float8_e4m3
    poses = f('poses')
    embed_w, embed_b = f('embed_w'), f('embed_b')
    ln0_g, ln0_b = f('ln0_g'), f('ln0_b')
    inw, inb = f('inw'), f('inb')
    outw, outb = f('outw'), f('outb')
    ln1g, ln1b = f('ln1g'), f('ln1b')
    ln2g, ln2b = f('ln2g'), f('ln2b')
    ff1w, ff1b = f('ff1w'), f('ff1b')
    ff2w, ff2b = f('ff2w'), f('ff2b')
    conv1w, conv1b = f('conv1w'), f('conv1b')
    bn1g, bn1b, bn1m, bn1v = f('bn1g'), f('bn1b'), f('bn1m'), f('bn1v')
    conv2w, conv2b = f('conv2w'), f('conv2b')
    bn2g, bn2b, bn2m, bn2v = f('bn2g'), f('bn2b'), f('bn2m'), f('bn2v')
    fc1w, fc1b = f('fc1w'), f('fc1b')
    fc2w, fc2b = f('fc2w'), f('fc2b')

    def moblk(w_t, nk, nm):
        # [nk*128, nm*128] -> [nm, 128, nk*128]: per-partition contiguous
        return np.ascontiguousarray(
            w_t.reshape(nk, P, nm, P).transpose(2, 1, 0, 3).reshape(nm, P, nk * P))

    def dbl8(ws):
        # fp8 double-quant at one scale: (hi, lo) with hi+lo ~= ws
        hi = ws.astype(f8)
        lo = (ws - hi.astype(np.float32)).astype(f8)
        return hi, lo

    def moblk_dbl(w_t_scaled, nk, nm):
        # [nm, 128, 2, nk*128] fp8 (hi, lo interleaved per mo tile)
        blk = moblk(w_t_scaled, nk, nm)
        hi, lo = dbl8(blk)
        return np.ascontiguousarray(
            np.stack([hi, lo], axis=2).reshape(nm, P, 2 * nk * P))

    shared = {}
    ewt = np.zeros((KIN, D), np.float32)
    ewt[:IN_DIM] = embed_w.T
    shared['emb_wt'] = moblk(ewt, 2, 4)
    shared['emb_b'] = embed_b
    shared['ln0_g'] = ln0_g

    qkv_wt = np.empty((NL, D, 3 * D), np.float32)
    qk_bf = np.empty((NL, 2 * D), np.float32)
    out_bf = np.empty((NL, D), np.float32)
    ff1_wtf = np.empty((NL, D, DFF), np.float32)
    ff1_bf = np.empty((NL, DFF), np.float32)
    ff2_wtf = np.empty((NL, DFF, D), np.float32)
    ff2_bf = np.empty((NL, D), np.float32)
    out_wtf = np.empty((NL, D, D), np.float32)
    for l in range(NL):
        w = inw[l]                      # [3D, D]
        qkv_wt[l] = (w * ln1g[l][None, :]).T
        qkv_bias = inb[l] + w @ ln1b[l]
        qk_bf[l] = qkv_bias[:2 * D] * SW
        out_wtf[l] = outw[l].T * SB
        out_bf[l] = outb[l] + outw[l] @ qkv_bias[2 * D:]
        ff1_wtf[l] = (ff1w[l] * ln2g[l][None, :]).T * SW
        ff1_bf[l] = (ff1b[l] + ff1w[l] @ ln2b[l]) * SW
        ff2_wtf[l] = ff2w[l].T * SB
        ff2_bf[l] = ff2b[l]
    # merged per-layer bias vector: [qk(8*128) | out(4*128) | ff1(16*128) |
    # ff2(4*128)] = 32*128 floats, loaded as one [128, 32] column tile
    shared['lbias'] = np.ascontiguousarray(
        np.concatenate([qk_bf, out_bf, ff1_bf, ff2_bf], axis=1))
    shared['q_wt'] = np.stack(
        [moblk(qkv_wt[l][:, 0:D] * SW, 4, 4) for l in range(NL)]).astype(f8)
    shared['k_wt'] = np.stack(
        [moblk(qkv_wt[l][:, D:2 * D] * SW, 4, 4) for l in range(NL)]).astype(f8)
    # V weights in rhs layout [NL, 2(hi/lo), ko, 128, 512]
    v_s = (qkv_wt[:, :, 2 * D:] * SW).reshape(NL, 4, P, D)
    v_hi, v_lo = dbl8(v_s)
    shared['v_wt'] = np.ascontiguousarray(
        np.stack([v_hi, v_lo], axis=1))
    shared['out_wt'] = np.stack(
        [moblk_dbl(out_wtf[l], 4, 4) for l in range(NL)])
    shared['ff1_wt'] = np.stack(
        [moblk_dbl(ff1_wtf[l], 4, 16) for l in range(NL)])
    shared['ff2_wt'] = np.stack(
        [moblk_dbl(ff2_wtf[l], 16, 4) for l in range(NL)])

    bn1sc = bn1g / np.sqrt(bn1v + EPS)
    bn2sc = bn2g / np.sqrt(bn2v + EPS)
    c1t = conv1w.transpose(2, 1, 0) * 0.5           # [5, D_in, D_out]
    shared['c1_wt'] = np.stack(
        [moblk(c1t[k], 4, 4) for k in range(5)]).astype(bf)
    shared['bn1_s'] = bn1sc
    shared['bn1_t'] = (conv1b - bn1m) * bn1sc + bn1b
    c2t = conv2w.transpose(2, 1, 0)
    shared['c2_wt'] = np.stack(
        [moblk(c2t[k], 4, 4) for k in range(3)]).astype(bf)
    shared['bn2_s'] = bn2sc
    shared['bn2_t'] = (conv2b - bn2m) * bn2sc + bn2b
    shared['fc1_wt'] = moblk(np.ascontiguousarray(fc1w.T), 4, 2).astype(bf)
    shared['fc1_b'] = fc1b
    f2 = np.zeros((D // 2, NCP), np.float32)
    f2[:, :NCLS] = fc2w.T
    shared['fc2_wt'] = moblk(f2, 2, 11).astype(bf)
    f2b = np.zeros((NCP,), np.float32)
    f2b[:NCLS] = fc2b
    shared['fc2_b'] = f2b

    inv = 1.0 / (10000.0 ** (np.arange(0, D, 2, dtype=np.float32) / D))
    si = np.arange(T, dtype=np.float32)[:, None] * inv[None, :]
    pos = np.stack([np.sin(si), np.cos(si)], -1).reshape(T, D)
    pos = pos.astype(np.float32)
    pos_t_g = (pos + ln0_b[None, :]).T.copy()       # [D, T]

    in_maps = []
    for c in range(8):
        b, h = c // 2, c % 2
        own = slice(h * TH, (h + 1) * TH)
        pt = np.zeros((KIN, TH), np.float32)
        pt[:IN_DIM] = poses[b, own].T
        edges_a = np.zeros((P, 2), np.float32)
        edges_a[:, 0] = 1.0 if h == 1 else 0.0
        edges_a[:, 1] = 1.0 if h == 0 else 0.0
        m = dict(shared)
        m['poses_t'] = pt
        m['pos_t'] = pos_t_g[:, own]
        m['edges'] = edges_a
        in_maps.append({k: np.ascontiguousarray(v) for k, v in m.items()})
    return in_maps


def _get_runner():
    """Build the module once and cache a jitted SPMD executable whose weight
    operands stay device-resident between calls."""
    if 'runner' in _CACHE:
        return _CACHE['runner']
    import jax
    import concourse.mybir as mybir_
    from concourse import bass2jax
    from jax.experimental.shard_map import shard_map
    from jax.sharding import Mesh, NamedSharding, PartitionSpec

    nc = _build()
    bass2jax.install_neuronx_cc_hook()
    partition_name = (nc.partition_id_tensor.name
                      if nc.partition_id_tensor else None)
    in_names, out_names, out_avals, zero_outs = [], [], [], []
    for alloc in nc.m.functions[0].allocations:
        if not isinstance(alloc, mybir_.MemoryLocationSet):
            continue
        name = alloc.memorylocations[0].name
        if alloc.kind == "ExternalInput":
            if name != partition_name:
                in_names.append(name)
        elif alloc.kind == "ExternalOutput":
            shape = tuple(alloc.tensor_shape)
            dtype = mybir_.dt.np(alloc.dtype)
            out_names.append(name)
            out_avals.append(jax.core.ShapedArray(shape, dtype))
            zero_outs.append((shape, dtype))
    n_params = len(in_names)
    all_names = in_names + out_names
    if partition_name is not None:
        all_names.append(partition_name)
    donate = tuple(range(n_params, n_params + len(out_names)))

    def _body(*args):
        operands = list(args)
        if partition_name is not None:
            operands.append(bass2jax.partition_id_tensor())
        outs = bass2jax._bass_exec_p.bind(
            *operands,
            out_avals=tuple(out_avals),
            in_names=tuple(all_names),
            out_names=tuple(out_names),
            lowering_input_output_aliases=(),
            sim_require_finite=True,
            sim_require_nnan=True,
            nc=nc,
        )
        return tuple(outs)

    devices = jax.devices()[:8]
    mesh = Mesh(np.asarray(devices), ("core",))
    spec = PartitionSpec("core")
    sharding = NamedSharding(mesh, spec)
    jitted = jax.jit(
        shard_map(_body, mesh=mesh, in_specs=(spec,) * (n_params + len(out_names)),
                  out_specs=(spec,) * len(out_names), check_rep=False),
        donate_argnums=donate, keep_unused=True)

    runner = dict(jitted=jitted, in_names=in_names, out_names=out_names,
                  zero_outs=zero_outs, sharding=sharding)
    _CACHE['runner'] = runner
    return runner


def _put_args(in_maps):
    import jax
    r = _get_runner()
    args = []
    for name in r['in_names']:
        concat = np.concatenate([in_maps[c][name] for c in range(8)], axis=0)
        args.append(jax.device_put(concat, r['sharding']))
    return args


def _exec(args):
    """Run with device-resident input args; returns per-core result dicts.
    Output (donated) buffers are freshly zero-allocated per call."""
    import jax
    r = _get_runner()
    outs_in = [jax.device_put(np.zeros((8 * s[0],) + s[1:], d), r['sharding'])
               for s, d in r['zero_outs']]
    outs = r['jitted'](*args, *outs_in)
    outs = [np.asarray(o) for o in outs]
    return [{name: outs[i].reshape(8, *r['zero_outs'][i][0])[c]
             for i, name in enumerate(r['out_names'])}
            for c in range(8)]


def _run(in_maps):
    return _exec(_put_args(in_maps))


def kernel(**inputs):
    in_maps = _prep_inputs(inputs)
    results = _run(in_maps)
    out = np.empty((B, T // 2, NCLS), np.float32)
    for c in range(8):
        b, h = c // 2, c % 2
        out[b, h * TP:(h + 1) * TP, :] = results[c]['out'][:NCLS].T
    return out
